# revision 18
# baseline (speedup 1.0000x reference)
"""Trainium2 Bass kernel for the nn_Dynamics problem (v2).

Math (per batch element, d=8, H=128):
  x = X[:, :8], v = X[:, 8:]
  h0 = tanh(W0 x + b0); h1 = tanh(W1 h0 + b1)
  A0 = W1^T (w2*(1-h1^2))  [via  c0 - (W1*w2) @ h1^2, c0 = W1^T w2]
  a0' = (h0^2-1)*A0 = -a0;  g' = W0^T a0' = -g
  h0p' = (h0^2-1)*t0 (t0 = W0 v); t1c = W1 h0p' = -t1
  e1' = (1-h1^2)*h1*t1c^2 = e1/w2;  w = h0*t0^2;  e2c = a0'*w = -A0*u
  hvv = sum_k -2*w2[k]*e1'[k] + 2*e2c[k]
  p = -(K x + D v)
  out = p + g'*(hvv - g'.p)/(1+|g|^2)   (Sherman-Morrison)

Layout: features on partitions, batch on the free axis. Supertiles of 1024
batch columns (2 matmul tiles of 512). X is transposed host-side and
shipped as strips at SBUF partitions 0:8 (x), 32:40 (v), 64:80 (x|v) so
stage A (z0/t0/force) runs as 3 concurrent row-tiled matmuls. g'/hv use
col-tiled matmuls into one fm bank-pair. The batch-major tail runs after
DMA-engine XBAR transposes (no PE transpose, no PSUM for the tail).

Engine split: Scalar {tanh x2, t0/A0 evac (+c0 bias), E copy}, Vector
{4 custom 1-src2-tensor ops, h1sq, e2-half, reciprocal}, GpSimd {e2-half,
tail}, PE {8 matmul streams/supertile}, DMA {in/out, 8 transposes}.

Sharding: pure data parallel over 8 NeuronCores (8192 rows each), weights
replicated, outputs concatenated.
"""

import os

import numpy as np

import concourse.bacc as bacc
import concourse.bass as bass
import concourse.dve_ops as dve_ops
import concourse.tile as tile
from concourse import mybir
from concourse.bass_utils import run_bass_kernel_spmd
from concourse.dve_ops import DveOp
from concourse.dve_ops import has_src1
from concourse.dve_spec import C0, One, Spec, Src0, Src1, lower, sq
from concourse.masks import make_identity

F32 = mybir.dt.float32
F16 = mybir.dt.float16
AX = mybir.AxisListType
OP = mybir.AluOpType
ACT = mybir.ActivationFunctionType

DIM = 8
H = 128
BATCH = 65536
NCORES = 8
BC = BATCH // NCORES          # 8192 rows per core
ST = 1024                     # supertile width (batch cols)
NST = BC // ST                # 8 supertiles
TW = 512                      # matmul tile width (one PSUM bank of f32)
NCH = ST // 128               # 8 transpose chunks per supertile

LAST_RESULTS = None

# ---------------- custom fused DVE ops ----------------


def _register_op(name, body, reference):
    if name in dve_ops._SUB_OPCODE_FOR_NAME:
        for op in dve_ops.OPS:
            if op.name == name:
                return op
    from concourse.dve_uop import DveOpSpec

    spec = Spec(body=body, reference=reference)
    shas = {}
    for ver in ("v3", "v4"):
        shas[ver] = DveOpSpec(
            name=name,
            opcode=dve_ops._CUSTOM_DVE_ROW_BASE + len(dve_ops.OPS),
            uops=lower(spec, ver=ver),
            rd1_en=has_src1(spec),
        ).sha(ver)
    op = DveOp(name, spec, subdim=False, uops_sha=shas)
    dve_ops.OPS.append(op)
    dve_ops.CUSTOM_DVE_SPECS[name] = spec
    dve_ops._SUB_OPCODE_FOR_NAME[name] = (
        dve_ops._CUSTOM_DVE_ROW_BASE + len(dve_ops.OPS) - 1
    )
    return op


# h0p' = (h0^2 - 1) * t0     (also a0' = (h0^2 - 1) * A0s)
OP_SQM1_MUL = _register_op(
    "ANT_SQM1_MUL",
    (sq(Src0) - One) * Src1,
    lambda in0, in1: (in0 * in0 - 1.0) * in1,
)
# w = h0 * t0^2
OP_WOP = _register_op(
    "ANT_WOP",
    Src0 * sq(Src1),
    lambda in0, in1: in0 * in1 * in1,
)
# a0' = (h0^2 - 1) * (A0n + c0)
OP_A0F = _register_op(
    "ANT_A0F",
    (sq(Src0) - One) * (Src1 + C0),
    lambda in0, in1, s0: (in0 * in0 - 1.0) * (in1 + s0),
)
# e1' = (1 - h1^2) * h1 * t1^2
OP_E1G = _register_op(
    "ANT_E1G",
    (One - sq(Src0)) * Src0 * sq(Src1),
    lambda in0, in1: (1.0 - in0 * in0) * in0 * in1 * in1,
)


def build_nc():
    nc = bacc.Bacc()

    XTd = nc.dram_tensor("XTd", [32, BC], F16, kind="ExternalInput")
    Wrow = nc.dram_tensor("Wrow", [80, H], F16, kind="ExternalInput")
    W1T = nc.dram_tensor("W1T", [H, H], F16, kind="ExternalInput")
    Wa = nc.dram_tensor("Wa", [H, H], F16, kind="ExternalInput")
    W0p = nc.dram_tensor("W0p", [H, 32], F16, kind="ExternalInput")
    hvW = nc.dram_tensor("hvW", [H, 64], F16, kind="ExternalInput")
    cb = nc.dram_tensor("cb", [H, 3], F32, kind="ExternalInput")
    out = nc.dram_tensor("out", [BC, DIM], F32, kind="ExternalOutput")

    from contextlib import ExitStack

    with tile.TileContext(nc) as tc, ExitStack() as stk:
        consts = stk.enter_context(tc.tile_pool(name="consts", bufs=1))
        work = stk.enter_context(tc.tile_pool(name="work", bufs=3))
        ps = stk.enter_context(tc.tile_pool(name="ps", bufs=1, space="PSUM"))

        # ---------------- constants ----------------
        XTX = consts.tile([8, BC], F16)
        XTV = consts.tile([8, BC], F16)
        XTF = consts.tile([16, BC], F16)
        # all strips at partition base 0 so every stage-A matmul runs in
        # plain 128x128 mode (no tiling-mode switches); x/v in quarter
        # chunks so supertile 0 starts early, f (force, only read by the
        # tail blocks) afterwards in halves.
        QB = BC // 4
        for h in range(4):
            cs = slice(h * QB, (h + 1) * QB)
            nc.sync.dma_start(out=XTX[:, cs], in_=XTd[0:8, cs])
            nc.sync.dma_start(out=XTV[:, cs], in_=XTd[8:16, cs])
        HB = BC // 2
        for h in range(2):
            cs = slice(h * HB, (h + 1) * HB)
            nc.sync.dma_start(out=XTF[:, cs], in_=XTd[16:32, cs])

        Wrow_sb = consts.tile([8, H], F16)
        nc.sync.dma_start(out=Wrow_sb, in_=Wrow[0:8, :])
        KDT_sb = consts.tile([16, DIM], F16)
        nc.sync.dma_start(out=KDT_sb, in_=Wrow[64:80, 0:DIM])
        W1T_sb = consts.tile([H, H], F16)
        nc.sync.dma_start(out=W1T_sb, in_=W1T[:, :])
        Wa_sb = consts.tile([H, H], F16)
        nc.sync.dma_start(out=Wa_sb, in_=Wa[:, :])
        W0p_sb = consts.tile([H, 32], F16)
        nc.sync.dma_start(out=W0p_sb, in_=W0p[:, :])
        hvW_sb = consts.tile([H, 64], F16)
        nc.sync.dma_start(out=hvW_sb, in_=hvW[:, :])
        cb_sb = consts.tile([H, 3], F32)
        nc.sync.dma_start(out=cb_sb, in_=cb[:, :])
        b0c = cb_sb[:, 0:1]
        b1c = cb_sb[:, 1:2]
        c0c = cb_sb[:, 2:3]

        identF = consts.tile([128, 128], F32)
        make_identity(nc, identF)
        identH = consts.tile([128, 128], F16)
        nc.scalar.copy(identH, identF)

        out_sb = consts.tile([128, NST * NCH * DIM], F32)

        # tail scratch: Q[p, chunk, grp, 10]; grp0 = [g^2 x8, 1, 0] -> den,
        # grp1 = [-g*p x8, hvv, 0] -> num. Cols 8/9 of grp0 and col 9 of
        # grp1 are preset once (1.0 / 0.0) and never overwritten.
        Q = consts.tile([128, NCH * 20], F32)
        nc.vector.memset(Q, 0.0)
        Qv = Q.rearrange("p (c g k) -> p c g k", g=2, k=10)
        nc.vector.memset(Qv[:, :, 0, 8:9], 1.0)

        # ---------------- main loop (software-pipelined emission) ----------------
        # front(s): z0/t0 MMs, tanh0, h0p, w, z1/t1 MMs, t1 evac, tanh1,
        #           h1sq, A0n MMs, a0, e1, e2
        # tail(s):  force MMs, g/hv col MMs, E copy, PE transposes, tl,
        #           batch-major tail math, out DMA
        # tail(s) is emitted after front(s+1) so chain-head ops of s+1 get
        # engine-queue priority over the long tail of s.
        state = {}

        def front(s):
            cols = [slice(s * ST + h * TW, s * ST + (h + 1) * TW) for h in range(2)]
            z = ps.tile([128, ST], F32, tag="z", name=f"z_{s}")
            t = ps.tile([128, ST], F32, tag="t", name=f"t_{s}")
            for h in range(2):
                lo = slice(h * TW, (h + 1) * TW)
                nc.tensor.matmul(
                    z[:, lo], Wrow_sb, XTX[:, cols[h]], start=True, stop=True
                )
                nc.tensor.matmul(
                    t[:, lo], Wrow_sb, XTV[:, cols[h]], start=True, stop=True
                )
            h0 = work.tile([128, ST], F16, tag="h0", name=f"h0_{s}")
            nc.scalar.activation(h0, z, ACT.Tanh, bias=b0c, scale=1.0)
            h0p = work.tile([128, ST], F16, tag="h0p", name=f"h0p_{s}")
            nc.vector._custom_dve(OP_SQM1_MUL, out=h0p, in0=h0, in1=t[:, :])
            w = work.tile([128, ST], F16, tag="w", name=f"w_{s}")
            nc.vector._custom_dve(OP_WOP, out=w, in0=h0, in1=t[:, :])

            z1 = ps.tile([128, ST], F32, tag="z", name=f"z1_{s}")
            for h in range(2):
                lo = slice(h * TW, (h + 1) * TW)
                nc.tensor.matmul(z1[:, lo], W1T_sb, h0[:, lo], start=True, stop=True)
            t1s = work.tile([128, ST], F16, tag="t1s", name=f"t1s_{s}")
            t1h = []
            for h in range(2):
                lo = slice(h * TW, (h + 1) * TW)
                t1 = ps.tile([128, TW], F32, tag="a", bufs=2, name=f"t1_{s}_{h}")
                nc.tensor.matmul(t1, W1T_sb, h0p[:, lo], start=True, stop=True)
                nc.scalar.copy(t1s[:, lo], t1)
            h1 = work.tile([128, ST], F16, tag="h1", name=f"h1_{s}")
            nc.scalar.activation(h1, z1, ACT.Tanh, bias=b1c, scale=1.0)
            h1sq = work.tile([128, ST], F16, tag="h1sq", name=f"h1sq_{s}")
            nc.vector.tensor_mul(h1sq, h1, h1)

            a0 = work.tile([128, ST], F16, tag="a0", name=f"a0_{s}")
            for h in range(2):
                lo = slice(h * TW, (h + 1) * TW)
                A0n = ps.tile([128, TW], F32, tag="a", bufs=2, name=f"A0n_{s}_{h}")
                nc.tensor.matmul(A0n, Wa_sb, h1sq[:, lo], start=True, stop=True)
                nc.vector._custom_dve(
                    OP_A0F, out=a0[:, lo], in0=h0[:, lo], in1=A0n, s0=c0c
                )
            e2 = work.tile([128, ST], F16, tag="e2", name=f"e2_{s}")
            nc.vector.tensor_mul(e2, a0, w)

            e1 = work.tile([128, ST], F16, tag="e1", name=f"e1_{s}")
            nc.vector._custom_dve(OP_E1G, out=e1, in0=h1, in1=t1s)
            state[s] = (cols, h0p, w, e1, a0, e2)

        def tail(s):
            cols, h0p, w, e1, a0, e2 = state.pop(s)
            fmh = []
            for h in range(2):
                lo = slice(h * TW, (h + 1) * TW)
                fm = ps.tile([128, TW], F32, tag="fm", bufs=2, name=f"fm_{s}_{h}")
                nc.tensor.matmul(
                    fm[0:8, :], KDT_sb, XTF[:, cols[h]], start=True, stop=True
                )
                nc.tensor.matmul(
                    fm[32:64, :], W0p_sb, a0[:, lo],
                    start=True, stop=True, tile_position=(0, 32),
                )
                nc.tensor.matmul(
                    fm[64:96, :], hvW_sb[:, 0:32], e1[:, lo],
                    start=True, stop=False, tile_position=(0, 64),
                )
                nc.tensor.matmul(
                    fm[64:96, :], hvW_sb[:, 32:64], e2[:, lo],
                    start=False, stop=True, tile_position=(0, 64),
                )
                fmh.append(fm)

            E = work.tile([128, ST], F16, tag="E", name=f"E_{s}")
            for h in range(2):
                lo = slice(h * TW, (h + 1) * TW)
                nc.scalar.copy(E[0:96, lo], fmh[h][0:96, :])
            bm = ps.tile([128, NCH * 96], F16, tag="a", bufs=2, name=f"bm_{s}")
            for c in range(NCH):
                nc.tensor.transpose(
                    bm[:, 96 * c : 96 * (c + 1)],
                    E[0:96, 128 * c : 128 * (c + 1)],
                    identH[0:96, 0:96],
                )
            tl = work.tile([128, NCH * 24], F32, tag="tl", name=f"tl_{s}")
            bmsrc = bass.AP(
                tensor=bm.tensor,
                offset=bm.offset,
                ap=[list(bm.ap[0]), [96, NCH], [32, 3], [1, DIM]],
            )
            tl4 = tl.rearrange("p (c q f) -> p c q f", q=3, f=DIM)
            nc.scalar.copy(tl4, bmsrc)

            p3 = tl4[:, :, 0, :]
            g3 = tl4[:, :, 1, :]
            hv1 = tl4[:, :, 2, 0:1]

            # strips hold -p, +g, -hvv (host-side sign flips), so the tail is
            # all plain ops: num' = -g.p - hvv = -num; su = g*(num'*rec) = -g*s
            nc.gpsimd.tensor_mul(Qv[:, :, 0, 0:8], g3, g3)
            nc.gpsimd.tensor_mul(Qv[:, :, 1, 0:8], g3, p3)
            nc.gpsimd.tensor_copy(Qv[:, :, 1, 8:9], hv1)
            R = work.tile([128, NCH * 2], F32, tag="R", name=f"R_{s}")
            Rv = R.rearrange("p (c g) -> p c g", g=2)
            nc.vector.tensor_reduce(Rv, Qv, axis=AX.X, op=OP.add)

            rec = work.tile([128, NCH], F32, tag="rec", name=f"rec_{s}")
            nc.vector.reciprocal(rec, Rv[:, :, 0:1].rearrange("p c g -> p (c g)"))
            s4 = work.tile([128, NCH], F32, tag="s4", name=f"s4_{s}")
            nc.vector.tensor_mul(
                s4, Rv[:, :, 1:2].rearrange("p c g -> p (c g)"), rec
            )
            s4b = bass.AP(
                tensor=s4.tensor,
                offset=s4.offset,
                ap=[list(s4.ap[0]), [1, NCH], [0, DIM]],
            )
            su = work.tile([128, NCH * DIM], F32, tag="su", name=f"su_{s}")
            su3 = su.rearrange("p (c f) -> p c f", f=DIM)
            nc.gpsimd.tensor_mul(su3, g3, s4b)
            ob = out_sb[:, NCH * DIM * s : NCH * DIM * (s + 1)]
            ob3 = ob.rearrange("p (c f) -> p c f", f=DIM)
            nc.gpsimd.tensor_sub(ob3, su3, p3)  # out = su - (-p)

            oap = out[:, :]
            dst = bass.AP(
                tensor=oap.tensor,
                offset=oap.offset + s * ST * DIM,
                ap=[[DIM, 128], [128 * DIM, NCH], [1, DIM]],
            )
            nc.sync.dma_start(out=dst, in_=ob3)

        for s in range(NST):
            front(s)
            if s > 0:
                tail(s - 1)
        tail(NST - 1)

    if not nc.is_finalized():
        nc.finalize()

    return nc


_NC_CACHE = None


def _install_ntff_shim():
    """Register the axon NTFF profile hook (missing antenv.axon_hooks shim)."""
    import sys
    import types

    if "antenv.axon_hooks" in sys.modules:
        return
    try:
        sys.path.insert(0, "/root/.axon_site")
        from trn_agent_boot.trn_boot import _ntff_profile_via_ctypes

        hook = _ntff_profile_via_ctypes("/opt/axon/libaxon_pjrt.so")
        mod = types.ModuleType("antenv.axon_hooks")
        mod.get_axon_ntff_profile_hook = lambda: hook
        sys.modules["antenv.axon_hooks"] = mod
    except Exception:
        pass


def kernel(**inputs):
    global LAST_RESULTS, _NC_CACHE
    trace = bool(int(os.environ.get("KERNEL_TRACE", "0")))
    if trace:
        _install_ntff_shim()
    if _NC_CACHE is None:
        _NC_CACHE = build_nc()
    nc = _NC_CACHE

    X = np.ascontiguousarray(inputs["X"], dtype=np.float32)
    K = np.asarray(inputs["K"], np.float32)
    D = np.asarray(inputs["D"], np.float32)
    W0 = np.asarray(inputs["W0"], np.float32)
    W1 = np.asarray(inputs["W1"], np.float32)
    W2 = np.asarray(inputs["W2"], np.float32)
    w2 = W2.reshape(H)

    wrow = np.zeros((80, H), np.float32)
    wrow[0:8] = W0.T
    wrow[32:40] = W0.T
    wrow[64:80, 0:8] = np.concatenate([K.T, D.T], axis=0)  # p-strip = -p
    wa = -(W1 * w2[:, None])
    w0p = np.zeros((H, 32), np.float32)
    w0p[:, 0:8] = -W0          # g-strip = +g (a0' is -a0)
    hvw = np.zeros((H, 64), np.float32)
    hvw[:, 0:8] = (2.0 * w2)[:, None]   # hv-strip = -hvv
    hvw[:, 32:40] = -2.0
    cbm = np.zeros((H, 3), np.float32)
    cbm[:, 0] = np.asarray(inputs["b0"], np.float32)
    cbm[:, 1] = np.asarray(inputs["b1"], np.float32)
    cbm[:, 2] = W1.T @ w2

    shared = {
        "Wrow": wrow.astype(np.float16),
        "W1T": np.ascontiguousarray(W1.T).astype(np.float16),
        "Wa": wa.astype(np.float16),
        "W0p": w0p.astype(np.float16),
        "hvW": hvw.astype(np.float16),
        "cb": cbm,
    }
    in_maps = []
    for i in range(NCORES):
        xc = X[i * BC : (i + 1) * BC]
        xt = np.ascontiguousarray(xc.T).astype(np.float16)  # [16, BC]
        xtd = np.concatenate([xt[0:8], xt[8:16], xt], axis=0)  # [32, BC]
        m = {"XTd": np.ascontiguousarray(xtd)}
        m.update(shared)
        in_maps.append(m)

    res = run_bass_kernel_spmd(
        nc, in_maps, core_ids=list(range(NCORES)), trace=trace
    )
    LAST_RESULTS = res
    out_full = np.concatenate([res.results[i]["out"] for i in range(NCORES)], axis=0)
    return out_full.astype(np.float32)


# revision 19
# speedup vs baseline: 1.0150x; 1.0150x over previous
"""Trainium2 Bass kernel for the nn_Dynamics problem (v2).

Math (per batch element, d=8, H=128):
  x = X[:, :8], v = X[:, 8:]
  h0 = tanh(W0 x + b0); h1 = tanh(W1 h0 + b1)
  A0 = W1^T (w2*(1-h1^2))  [via  c0 - (W1*w2) @ h1^2, c0 = W1^T w2]
  a0' = (h0^2-1)*A0 = -a0;  g' = W0^T a0' = -g
  h0p' = (h0^2-1)*t0 (t0 = W0 v); t1c = W1 h0p' = -t1
  e1' = (1-h1^2)*h1*t1c^2 = e1/w2;  w = h0*t0^2;  e2c = a0'*w = -A0*u
  hvv = sum_k -2*w2[k]*e1'[k] + 2*e2c[k]
  p = -(K x + D v)
  out = p + g'*(hvv - g'.p)/(1+|g|^2)   (Sherman-Morrison)

Layout: features on partitions, batch on the free axis. Supertiles of 1024
batch columns (2 matmul tiles of 512). X is transposed host-side and
shipped as strips at SBUF partitions 0:8 (x), 32:40 (v), 64:80 (x|v) so
stage A (z0/t0/force) runs as 3 concurrent row-tiled matmuls. g'/hv use
col-tiled matmuls into one fm bank-pair. The batch-major tail runs after
DMA-engine XBAR transposes (no PE transpose, no PSUM for the tail).

Engine split: Scalar {tanh x2, t0/A0 evac (+c0 bias), E copy}, Vector
{4 custom 1-src2-tensor ops, h1sq, e2-half, reciprocal}, GpSimd {e2-half,
tail}, PE {8 matmul streams/supertile}, DMA {in/out, 8 transposes}.

Sharding: pure data parallel over 8 NeuronCores (8192 rows each), weights
replicated, outputs concatenated.
"""

import os

import numpy as np

import concourse.bacc as bacc
import concourse.bass as bass
import concourse.dve_ops as dve_ops
import concourse.tile as tile
from concourse import mybir
from concourse.bass_utils import run_bass_kernel_spmd
from concourse.dve_ops import DveOp
from concourse.dve_ops import has_src1
from concourse.dve_spec import C0, One, Spec, Src0, Src1, lower, sq
from concourse.masks import make_identity

F32 = mybir.dt.float32
F16 = mybir.dt.float16
AX = mybir.AxisListType
OP = mybir.AluOpType
ACT = mybir.ActivationFunctionType

DIM = 8
H = 128
BATCH = 65536
NCORES = 8
BC = BATCH // NCORES          # 8192 rows per core
ST = 1024                     # supertile width (batch cols)
NST = BC // ST                # 8 supertiles
TW = 512                      # matmul tile width (one PSUM bank of f32)
NCH = ST // 128               # 8 transpose chunks per supertile

LAST_RESULTS = None

# ---------------- custom fused DVE ops ----------------


def _register_op(name, body, reference):
    if name in dve_ops._SUB_OPCODE_FOR_NAME:
        for op in dve_ops.OPS:
            if op.name == name:
                return op
    from concourse.dve_uop import DveOpSpec

    spec = Spec(body=body, reference=reference)
    shas = {}
    for ver in ("v3", "v4"):
        shas[ver] = DveOpSpec(
            name=name,
            opcode=dve_ops._CUSTOM_DVE_ROW_BASE + len(dve_ops.OPS),
            uops=lower(spec, ver=ver),
            rd1_en=has_src1(spec),
        ).sha(ver)
    op = DveOp(name, spec, subdim=False, uops_sha=shas)
    dve_ops.OPS.append(op)
    dve_ops.CUSTOM_DVE_SPECS[name] = spec
    dve_ops._SUB_OPCODE_FOR_NAME[name] = (
        dve_ops._CUSTOM_DVE_ROW_BASE + len(dve_ops.OPS) - 1
    )
    return op


# h0p' = (h0^2 - 1) * t0     (also a0' = (h0^2 - 1) * A0s)
OP_SQM1_MUL = _register_op(
    "ANT_SQM1_MUL",
    (sq(Src0) - One) * Src1,
    lambda in0, in1: (in0 * in0 - 1.0) * in1,
)
# w = h0 * t0^2
OP_WOP = _register_op(
    "ANT_WOP",
    Src0 * sq(Src1),
    lambda in0, in1: in0 * in1 * in1,
)
# a0' = (h0^2 - 1) * (A0n + c0)
OP_A0F = _register_op(
    "ANT_A0F",
    (sq(Src0) - One) * (Src1 + C0),
    lambda in0, in1, s0: (in0 * in0 - 1.0) * (in1 + s0),
)
# e1' = (1 - h1^2) * h1 * t1^2
OP_E1G = _register_op(
    "ANT_E1G",
    (One - sq(Src0)) * Src0 * sq(Src1),
    lambda in0, in1: (1.0 - in0 * in0) * in0 * in1 * in1,
)


def build_nc():
    nc = bacc.Bacc()

    XTd = nc.dram_tensor("XTd", [32, BC], F16, kind="ExternalInput")
    Wrow = nc.dram_tensor("Wrow", [80, H], F16, kind="ExternalInput")
    W1T = nc.dram_tensor("W1T", [H, H], F16, kind="ExternalInput")
    Wa = nc.dram_tensor("Wa", [H, H], F16, kind="ExternalInput")
    W0p = nc.dram_tensor("W0p", [H, 32], F16, kind="ExternalInput")
    hvW = nc.dram_tensor("hvW", [H, 64], F16, kind="ExternalInput")
    cb = nc.dram_tensor("cb", [H, 3], F32, kind="ExternalInput")
    out = nc.dram_tensor("out", [BC, DIM], F32, kind="ExternalOutput")

    from contextlib import ExitStack

    with tile.TileContext(nc) as tc, ExitStack() as stk:
        consts = stk.enter_context(tc.tile_pool(name="consts", bufs=1))
        work = stk.enter_context(tc.tile_pool(name="work", bufs=3))
        ps = stk.enter_context(tc.tile_pool(name="ps", bufs=1, space="PSUM"))

        # ---------------- constants ----------------
        XTX = consts.tile([8, BC], F16)
        XTV = consts.tile([8, BC], F16)
        XTF = consts.tile([16, BC], F16)
        # all strips at partition base 0 so every stage-A matmul runs in
        # plain 128x128 mode (no tiling-mode switches); two column halves
        # each so the first supertile starts early; f (force, only read by
        # the tail blocks) last.
        HB = BC // 2
        for h in range(2):
            cs = slice(h * HB, (h + 1) * HB)
            nc.sync.dma_start(out=XTX[:, cs], in_=XTd[0:8, cs])
            nc.sync.dma_start(out=XTV[:, cs], in_=XTd[8:16, cs])
        for h in range(2):
            cs = slice(h * HB, (h + 1) * HB)
            nc.sync.dma_start(out=XTF[:, cs], in_=XTd[16:32, cs])

        Wrow_sb = consts.tile([8, H], F16)
        nc.sync.dma_start(out=Wrow_sb, in_=Wrow[0:8, :])
        KDT_sb = consts.tile([16, DIM], F16)
        nc.sync.dma_start(out=KDT_sb, in_=Wrow[64:80, 0:DIM])
        W1T_sb = consts.tile([H, H], F16)
        nc.sync.dma_start(out=W1T_sb, in_=W1T[:, :])
        Wa_sb = consts.tile([H, H], F16)
        nc.sync.dma_start(out=Wa_sb, in_=Wa[:, :])
        W0p_sb = consts.tile([H, 32], F16)
        nc.sync.dma_start(out=W0p_sb, in_=W0p[:, :])
        hvW_sb = consts.tile([H, 64], F16)
        nc.sync.dma_start(out=hvW_sb, in_=hvW[:, :])
        cb_sb = consts.tile([H, 3], F32)
        nc.sync.dma_start(out=cb_sb, in_=cb[:, :])
        b0c = cb_sb[:, 0:1]
        b1c = cb_sb[:, 1:2]
        c0c = cb_sb[:, 2:3]

        identF = consts.tile([128, 128], F32)
        make_identity(nc, identF)
        identH = consts.tile([128, 128], F16)
        nc.scalar.copy(identH, identF)

        out_sb = consts.tile([128, NST * NCH * DIM], F32)

        # tail scratch: Q[p, chunk, grp, 10]; grp0 = [g^2 x8, 1, 0] -> den,
        # grp1 = [-g*p x8, hvv, 0] -> num. Cols 8/9 of grp0 and col 9 of
        # grp1 are preset once (1.0 / 0.0) and never overwritten.
        Q = consts.tile([128, NCH * 20], F32)
        nc.vector.memset(Q, 0.0)
        Qv = Q.rearrange("p (c g k) -> p c g k", g=2, k=10)
        nc.vector.memset(Qv[:, :, 0, 8:9], 1.0)

        # ---------------- main loop (software-pipelined emission) ----------------
        # front(s): z0/t0 MMs, tanh0, h0p, w, z1/t1 MMs, t1 evac, tanh1,
        #           h1sq, A0n MMs, a0, e1, e2
        # tail(s):  force MMs, g/hv col MMs, E copy, PE transposes, tl,
        #           batch-major tail math, out DMA
        # tail(s) is emitted after front(s+1) so chain-head ops of s+1 get
        # engine-queue priority over the long tail of s.
        state = {}

        def front(s):
            cols = [slice(s * ST + h * TW, s * ST + (h + 1) * TW) for h in range(2)]
            z = ps.tile([128, ST], F32, tag="z", name=f"z_{s}")
            t = ps.tile([128, ST], F32, tag="t", name=f"t_{s}")
            for h in range(2):
                lo = slice(h * TW, (h + 1) * TW)
                nc.tensor.matmul(
                    z[:, lo], Wrow_sb, XTX[:, cols[h]], start=True, stop=True
                )
                nc.tensor.matmul(
                    t[:, lo], Wrow_sb, XTV[:, cols[h]], start=True, stop=True
                )
            h0 = work.tile([128, ST], F16, tag="h0", name=f"h0_{s}")
            nc.scalar.activation(h0, z, ACT.Tanh, bias=b0c, scale=1.0)
            h0p = work.tile([128, ST], F16, tag="h0p", name=f"h0p_{s}")
            nc.vector._custom_dve(OP_SQM1_MUL, out=h0p, in0=h0, in1=t[:, :])
            w = work.tile([128, ST], F16, tag="w", name=f"w_{s}")
            nc.vector._custom_dve(OP_WOP, out=w, in0=h0, in1=t[:, :])

            z1 = ps.tile([128, ST], F32, tag="z", name=f"z1_{s}")
            for h in range(2):
                lo = slice(h * TW, (h + 1) * TW)
                nc.tensor.matmul(z1[:, lo], W1T_sb, h0[:, lo], start=True, stop=True)
            t1s = work.tile([128, ST], F16, tag="t1s", name=f"t1s_{s}")
            t1h = []
            for h in range(2):
                lo = slice(h * TW, (h + 1) * TW)
                t1 = ps.tile([128, TW], F32, tag="a", bufs=2, name=f"t1_{s}_{h}")
                nc.tensor.matmul(t1, W1T_sb, h0p[:, lo], start=True, stop=True)
                nc.scalar.copy(t1s[:, lo], t1)
            h1 = work.tile([128, ST], F16, tag="h1", name=f"h1_{s}")
            nc.scalar.activation(h1, z1, ACT.Tanh, bias=b1c, scale=1.0)
            h1sq = work.tile([128, ST], F16, tag="h1sq", name=f"h1sq_{s}")
            nc.vector.tensor_mul(h1sq, h1, h1)

            a0 = work.tile([128, ST], F16, tag="a0", name=f"a0_{s}")
            for h in range(2):
                lo = slice(h * TW, (h + 1) * TW)
                A0n = ps.tile([128, TW], F32, tag="a", bufs=2, name=f"A0n_{s}_{h}")
                nc.tensor.matmul(A0n, Wa_sb, h1sq[:, lo], start=True, stop=True)
                nc.vector._custom_dve(
                    OP_A0F, out=a0[:, lo], in0=h0[:, lo], in1=A0n, s0=c0c
                )
            e2 = work.tile([128, ST], F16, tag="e2", name=f"e2_{s}")
            nc.vector.tensor_mul(e2, a0, w)

            e1 = work.tile([128, ST], F16, tag="e1", name=f"e1_{s}")
            nc.vector._custom_dve(OP_E1G, out=e1, in0=h1, in1=t1s)
            state[s] = (cols, h0p, w, e1, a0, e2)

        def tail(s):
            cols, h0p, w, e1, a0, e2 = state.pop(s)
            fmh = []
            for h in range(2):
                lo = slice(h * TW, (h + 1) * TW)
                fm = ps.tile([128, TW], F32, tag="fm", bufs=2, name=f"fm_{s}_{h}")
                nc.tensor.matmul(
                    fm[0:8, :], KDT_sb, XTF[:, cols[h]], start=True, stop=True
                )
                nc.tensor.matmul(
                    fm[32:64, :], W0p_sb, a0[:, lo],
                    start=True, stop=True, tile_position=(0, 32),
                )
                nc.tensor.matmul(
                    fm[64:96, :], hvW_sb[:, 0:32], e1[:, lo],
                    start=True, stop=False, tile_position=(0, 64),
                )
                nc.tensor.matmul(
                    fm[64:96, :], hvW_sb[:, 32:64], e2[:, lo],
                    start=False, stop=True, tile_position=(0, 64),
                )
                fmh.append(fm)

            E = work.tile([128, ST], F16, tag="E", name=f"E_{s}")
            for h in range(2):
                lo = slice(h * TW, (h + 1) * TW)
                nc.scalar.copy(E[0:96, lo], fmh[h][0:96, :])
            bm = ps.tile([128, NCH * 96], F16, tag="a", bufs=2, name=f"bm_{s}")
            for c in range(NCH):
                nc.tensor.transpose(
                    bm[:, 96 * c : 96 * (c + 1)],
                    E[0:96, 128 * c : 128 * (c + 1)],
                    identH[0:96, 0:96],
                )
            tl = work.tile([128, NCH * 24], F32, tag="tl", name=f"tl_{s}")
            bmsrc = bass.AP(
                tensor=bm.tensor,
                offset=bm.offset,
                ap=[list(bm.ap[0]), [96, NCH], [32, 3], [1, DIM]],
            )
            tl4 = tl.rearrange("p (c q f) -> p c q f", q=3, f=DIM)
            nc.scalar.copy(tl4, bmsrc)

            p3 = tl4[:, :, 0, :]
            g3 = tl4[:, :, 1, :]
            hv1 = tl4[:, :, 2, 0:1]

            # strips hold -p, +g, -hvv (host-side sign flips), so the tail is
            # all plain ops: num' = -g.p - hvv = -num; su = g*(num'*rec) = -g*s
            nc.gpsimd.tensor_mul(Qv[:, :, 0, 0:8], g3, g3)
            nc.gpsimd.tensor_mul(Qv[:, :, 1, 0:8], g3, p3)
            nc.gpsimd.tensor_copy(Qv[:, :, 1, 8:9], hv1)
            R = work.tile([128, NCH * 2], F32, tag="R", name=f"R_{s}")
            Rv = R.rearrange("p (c g) -> p c g", g=2)
            nc.vector.tensor_reduce(Rv, Qv, axis=AX.X, op=OP.add)

            rec = work.tile([128, NCH], F32, tag="rec", name=f"rec_{s}")
            nc.vector.reciprocal(rec, Rv[:, :, 0:1].rearrange("p c g -> p (c g)"))
            s4 = work.tile([128, NCH], F32, tag="s4", name=f"s4_{s}")
            nc.vector.tensor_mul(
                s4, Rv[:, :, 1:2].rearrange("p c g -> p (c g)"), rec
            )
            s4b = bass.AP(
                tensor=s4.tensor,
                offset=s4.offset,
                ap=[list(s4.ap[0]), [1, NCH], [0, DIM]],
            )
            su = work.tile([128, NCH * DIM], F32, tag="su", name=f"su_{s}")
            su3 = su.rearrange("p (c f) -> p c f", f=DIM)
            nc.gpsimd.tensor_mul(su3, g3, s4b)
            ob = out_sb[:, NCH * DIM * s : NCH * DIM * (s + 1)]
            ob3 = ob.rearrange("p (c f) -> p c f", f=DIM)
            nc.gpsimd.tensor_sub(ob3, su3, p3)  # out = su - (-p)

            oap = out[:, :]
            dst = bass.AP(
                tensor=oap.tensor,
                offset=oap.offset + s * ST * DIM,
                ap=[[DIM, 128], [128 * DIM, NCH], [1, DIM]],
            )
            nc.sync.dma_start(out=dst, in_=ob3)

        for s in range(NST):
            front(s)
            if s > 0:
                tail(s - 1)
        tail(NST - 1)

    if not nc.is_finalized():
        nc.finalize()

    return nc


_NC_CACHE = None


def _install_ntff_shim():
    """Register the axon NTFF profile hook (missing antenv.axon_hooks shim)."""
    import sys
    import types

    if "antenv.axon_hooks" in sys.modules:
        return
    try:
        sys.path.insert(0, "/root/.axon_site")
        from trn_agent_boot.trn_boot import _ntff_profile_via_ctypes

        hook = _ntff_profile_via_ctypes("/opt/axon/libaxon_pjrt.so")
        mod = types.ModuleType("antenv.axon_hooks")
        mod.get_axon_ntff_profile_hook = lambda: hook
        sys.modules["antenv.axon_hooks"] = mod
    except Exception:
        pass


def kernel(**inputs):
    global LAST_RESULTS, _NC_CACHE
    trace = bool(int(os.environ.get("KERNEL_TRACE", "0")))
    if trace:
        _install_ntff_shim()
    if _NC_CACHE is None:
        _NC_CACHE = build_nc()
    nc = _NC_CACHE

    X = np.ascontiguousarray(inputs["X"], dtype=np.float32)
    K = np.asarray(inputs["K"], np.float32)
    D = np.asarray(inputs["D"], np.float32)
    W0 = np.asarray(inputs["W0"], np.float32)
    W1 = np.asarray(inputs["W1"], np.float32)
    W2 = np.asarray(inputs["W2"], np.float32)
    w2 = W2.reshape(H)

    wrow = np.zeros((80, H), np.float32)
    wrow[0:8] = W0.T
    wrow[32:40] = W0.T
    wrow[64:80, 0:8] = np.concatenate([K.T, D.T], axis=0)  # p-strip = -p
    wa = -(W1 * w2[:, None])
    w0p = np.zeros((H, 32), np.float32)
    w0p[:, 0:8] = -W0          # g-strip = +g (a0' is -a0)
    hvw = np.zeros((H, 64), np.float32)
    hvw[:, 0:8] = (2.0 * w2)[:, None]   # hv-strip = -hvv
    hvw[:, 32:40] = -2.0
    cbm = np.zeros((H, 3), np.float32)
    cbm[:, 0] = np.asarray(inputs["b0"], np.float32)
    cbm[:, 1] = np.asarray(inputs["b1"], np.float32)
    cbm[:, 2] = W1.T @ w2

    shared = {
        "Wrow": wrow.astype(np.float16),
        "W1T": np.ascontiguousarray(W1.T).astype(np.float16),
        "Wa": wa.astype(np.float16),
        "W0p": w0p.astype(np.float16),
        "hvW": hvw.astype(np.float16),
        "cb": cbm,
    }
    in_maps = []
    for i in range(NCORES):
        xc = X[i * BC : (i + 1) * BC]
        xt = np.ascontiguousarray(xc.T).astype(np.float16)  # [16, BC]
        xtd = np.concatenate([xt[0:8], xt[8:16], xt], axis=0)  # [32, BC]
        m = {"XTd": np.ascontiguousarray(xtd)}
        m.update(shared)
        in_maps.append(m)

    res = run_bass_kernel_spmd(
        nc, in_maps, core_ids=list(range(NCORES)), trace=trace
    )
    LAST_RESULTS = res
    out_full = np.concatenate([res.results[i]["out"] for i in range(NCORES)], axis=0)
    return out_full.astype(np.float32)


# revision 20
# speedup vs baseline: 1.0254x; 1.0102x over previous
"""Trainium2 Bass kernel for the nn_Dynamics problem (v2).

Math (per batch element, d=8, H=128):
  x = X[:, :8], v = X[:, 8:]
  h0 = tanh(W0 x + b0); h1 = tanh(W1 h0 + b1)
  A0 = W1^T (w2*(1-h1^2))  [via  c0 - (W1*w2) @ h1^2, c0 = W1^T w2]
  a0' = (h0^2-1)*A0 = -a0;  g' = W0^T a0' = -g
  h0p' = (h0^2-1)*t0 (t0 = W0 v); t1c = W1 h0p' = -t1
  e1' = (1-h1^2)*h1*t1c^2 = e1/w2;  w = h0*t0^2;  e2c = a0'*w = -A0*u
  hvv = sum_k -2*w2[k]*e1'[k] + 2*e2c[k]
  p = -(K x + D v)
  out = p + g'*(hvv - g'.p)/(1+|g|^2)   (Sherman-Morrison)

Layout: features on partitions, batch on the free axis. Supertiles of 1024
batch columns (2 matmul tiles of 512). X is transposed host-side and
shipped as strips at SBUF partitions 0:8 (x), 32:40 (v), 64:80 (x|v) so
stage A (z0/t0/force) runs as 3 concurrent row-tiled matmuls. g'/hv use
col-tiled matmuls into one fm bank-pair. The batch-major tail runs after
DMA-engine XBAR transposes (no PE transpose, no PSUM for the tail).

Engine split: Scalar {tanh x2, t0/A0 evac (+c0 bias), E copy}, Vector
{4 custom 1-src2-tensor ops, h1sq, e2-half, reciprocal}, GpSimd {e2-half,
tail}, PE {8 matmul streams/supertile}, DMA {in/out, 8 transposes}.

Sharding: pure data parallel over 8 NeuronCores (8192 rows each), weights
replicated, outputs concatenated.
"""

import os

import numpy as np

import concourse.bacc as bacc
import concourse.bass as bass
import concourse.dve_ops as dve_ops
import concourse.tile as tile
from concourse import mybir
from concourse.bass_utils import run_bass_kernel_spmd
from concourse.dve_ops import DveOp
from concourse.dve_ops import has_src1
from concourse.dve_spec import C0, One, Spec, Src0, Src1, lower, sq
from concourse.masks import make_identity

F32 = mybir.dt.float32
F16 = mybir.dt.float16
AX = mybir.AxisListType
OP = mybir.AluOpType
ACT = mybir.ActivationFunctionType

DIM = 8
H = 128
BATCH = 65536
NCORES = 8
BC = BATCH // NCORES          # 8192 rows per core
ST = 1024                     # supertile width (batch cols)
NST = BC // ST                # 8 supertiles
TW = 512                      # matmul tile width (one PSUM bank of f32)
NCH = ST // 128               # 8 transpose chunks per supertile

LAST_RESULTS = None

# ---------------- custom fused DVE ops ----------------


def _register_op(name, body, reference):
    if name in dve_ops._SUB_OPCODE_FOR_NAME:
        for op in dve_ops.OPS:
            if op.name == name:
                return op
    from concourse.dve_uop import DveOpSpec

    spec = Spec(body=body, reference=reference)
    shas = {}
    for ver in ("v3", "v4"):
        shas[ver] = DveOpSpec(
            name=name,
            opcode=dve_ops._CUSTOM_DVE_ROW_BASE + len(dve_ops.OPS),
            uops=lower(spec, ver=ver),
            rd1_en=has_src1(spec),
        ).sha(ver)
    op = DveOp(name, spec, subdim=False, uops_sha=shas)
    dve_ops.OPS.append(op)
    dve_ops.CUSTOM_DVE_SPECS[name] = spec
    dve_ops._SUB_OPCODE_FOR_NAME[name] = (
        dve_ops._CUSTOM_DVE_ROW_BASE + len(dve_ops.OPS) - 1
    )
    return op


# h0p' = (h0^2 - 1) * t0     (also a0' = (h0^2 - 1) * A0s)
OP_SQM1_MUL = _register_op(
    "ANT_SQM1_MUL",
    (sq(Src0) - One) * Src1,
    lambda in0, in1: (in0 * in0 - 1.0) * in1,
)
# w = h0 * t0^2
OP_WOP = _register_op(
    "ANT_WOP",
    Src0 * sq(Src1),
    lambda in0, in1: in0 * in1 * in1,
)
# a0' = (h0^2 - 1) * (A0n + c0)
OP_A0F = _register_op(
    "ANT_A0F",
    (sq(Src0) - One) * (Src1 + C0),
    lambda in0, in1, s0: (in0 * in0 - 1.0) * (in1 + s0),
)
# e1' = (1 - h1^2) * h1 * t1^2
OP_E1G = _register_op(
    "ANT_E1G",
    (One - sq(Src0)) * Src0 * sq(Src1),
    lambda in0, in1: (1.0 - in0 * in0) * in0 * in1 * in1,
)


def build_nc():
    nc = bacc.Bacc()

    XTd = nc.dram_tensor("XTd", [32, BC], F16, kind="ExternalInput")
    Wrow = nc.dram_tensor("Wrow", [80, H], F16, kind="ExternalInput")
    W1T = nc.dram_tensor("W1T", [H, H], F16, kind="ExternalInput")
    Wa = nc.dram_tensor("Wa", [H, H], F16, kind="ExternalInput")
    W0p = nc.dram_tensor("W0p", [H, 32], F16, kind="ExternalInput")
    hvW = nc.dram_tensor("hvW", [H, 64], F16, kind="ExternalInput")
    cb = nc.dram_tensor("cb", [H, 3], F32, kind="ExternalInput")
    out = nc.dram_tensor("out", [BC, DIM], F32, kind="ExternalOutput")

    from contextlib import ExitStack

    with tile.TileContext(nc) as tc, ExitStack() as stk:
        consts = stk.enter_context(tc.tile_pool(name="consts", bufs=1))
        work = stk.enter_context(tc.tile_pool(name="work", bufs=2))
        ps = stk.enter_context(tc.tile_pool(name="ps", bufs=1, space="PSUM"))

        # ---------------- constants ----------------
        XTX = consts.tile([8, BC], F16)
        XTV = consts.tile([8, BC], F16)
        XTF = consts.tile([16, BC], F16)
        # all strips at partition base 0 so every stage-A matmul runs in
        # plain 128x128 mode (no tiling-mode switches); two column halves
        # each so the first supertile starts early; f (force, only read by
        # the tail blocks) last.
        HB = BC // 2
        for h in range(2):
            cs = slice(h * HB, (h + 1) * HB)
            nc.sync.dma_start(out=XTX[:, cs], in_=XTd[0:8, cs])
            nc.sync.dma_start(out=XTV[:, cs], in_=XTd[8:16, cs])
        for h in range(2):
            cs = slice(h * HB, (h + 1) * HB)
            nc.sync.dma_start(out=XTF[:, cs], in_=XTd[16:32, cs])

        Wrow_sb = consts.tile([8, H], F16)
        nc.sync.dma_start(out=Wrow_sb, in_=Wrow[0:8, :])
        KDT_sb = consts.tile([16, DIM], F16)
        nc.sync.dma_start(out=KDT_sb, in_=Wrow[64:80, 0:DIM])
        W1T_sb = consts.tile([H, H], F16)
        nc.sync.dma_start(out=W1T_sb, in_=W1T[:, :])
        Wa_sb = consts.tile([H, H], F16)
        nc.sync.dma_start(out=Wa_sb, in_=Wa[:, :])
        W0p_sb = consts.tile([H, 32], F16)
        nc.sync.dma_start(out=W0p_sb, in_=W0p[:, :])
        hvW_sb = consts.tile([H, 64], F16)
        nc.sync.dma_start(out=hvW_sb, in_=hvW[:, :])
        cb_sb = consts.tile([H, 3], F32)
        nc.sync.dma_start(out=cb_sb, in_=cb[:, :])
        b0c = cb_sb[:, 0:1]
        b1c = cb_sb[:, 1:2]
        c0c = cb_sb[:, 2:3]

        identF = consts.tile([128, 128], F32)
        make_identity(nc, identF)
        identH = consts.tile([128, 128], F16)
        nc.scalar.copy(identH, identF)

        out_sb = consts.tile([128, NST * NCH * DIM], F32)

        # tail scratch: Q[p, chunk, grp, 10]; grp0 = [g^2 x8, 1, 0] -> den,
        # grp1 = [-g*p x8, hvv, 0] -> num. Cols 8/9 of grp0 and col 9 of
        # grp1 are preset once (1.0 / 0.0) and never overwritten.
        Q = consts.tile([128, NCH * 20], F32)
        nc.vector.memset(Q, 0.0)
        Qv = Q.rearrange("p (c g k) -> p c g k", g=2, k=10)
        nc.vector.memset(Qv[:, :, 0, 8:9], 1.0)

        # ---------------- main loop (software-pipelined emission) ----------------
        # front(s): z0/t0 MMs, tanh0, h0p, w, z1/t1 MMs, t1 evac, tanh1,
        #           h1sq, A0n MMs, a0, e1, e2
        # tail(s):  force MMs, g/hv col MMs, E copy, PE transposes, tl,
        #           batch-major tail math, out DMA
        # tail(s) is emitted after front(s+1) so chain-head ops of s+1 get
        # engine-queue priority over the long tail of s.
        state = {}

        def front(s):
            cols = [slice(s * ST + h * TW, s * ST + (h + 1) * TW) for h in range(2)]
            z = ps.tile([128, ST], F32, tag="z", name=f"z_{s}")
            t = ps.tile([128, ST], F32, tag="t", name=f"t_{s}")
            for h in range(2):
                lo = slice(h * TW, (h + 1) * TW)
                nc.tensor.matmul(
                    z[:, lo], Wrow_sb, XTX[:, cols[h]], start=True, stop=True
                )
                nc.tensor.matmul(
                    t[:, lo], Wrow_sb, XTV[:, cols[h]], start=True, stop=True
                )
            h0 = work.tile([128, ST], F16, tag="h0", name=f"h0_{s}")
            nc.scalar.activation(h0, z, ACT.Tanh, bias=b0c, scale=1.0)
            h0p = work.tile([128, ST], F16, tag="h0p", name=f"h0p_{s}")
            nc.vector._custom_dve(OP_SQM1_MUL, out=h0p, in0=h0, in1=t[:, :])
            w = work.tile([128, ST], F16, tag="w", name=f"w_{s}")
            nc.vector._custom_dve(OP_WOP, out=w, in0=h0, in1=t[:, :])

            z1 = ps.tile([128, ST], F32, tag="z", name=f"z1_{s}")
            for h in range(2):
                lo = slice(h * TW, (h + 1) * TW)
                nc.tensor.matmul(z1[:, lo], W1T_sb, h0[:, lo], start=True, stop=True)
            t1h = []
            for h in range(2):
                lo = slice(h * TW, (h + 1) * TW)
                t1 = ps.tile([128, TW], F32, tag="a", bufs=2, name=f"t1_{s}_{h}")
                nc.tensor.matmul(t1, W1T_sb, h0p[:, lo], start=True, stop=True)
                t1h.append(t1)
            h1 = work.tile([128, ST], F16, tag="h1", name=f"h1_{s}")
            nc.scalar.activation(h1, z1, ACT.Tanh, bias=b1c, scale=1.0)
            h1sq = work.tile([128, ST], F16, tag="h1sq", name=f"h1sq_{s}")
            nc.scalar.square(h1sq, h1)

            # e1 per half, before a0, so the A0n allocs (same PSUM tag) don't
            # wait behind a full-width e1
            e1 = work.tile([128, ST], F16, tag="e1", name=f"e1_{s}")
            for h in range(2):
                lo = slice(h * TW, (h + 1) * TW)
                nc.vector._custom_dve(
                    OP_E1G, out=e1[:, lo], in0=h1[:, lo], in1=t1h[h]
                )

            a0 = work.tile([128, ST], F16, tag="a0", name=f"a0_{s}")
            for h in range(2):
                lo = slice(h * TW, (h + 1) * TW)
                A0n = ps.tile([128, TW], F32, tag="a", bufs=2, name=f"A0n_{s}_{h}")
                nc.tensor.matmul(A0n, Wa_sb, h1sq[:, lo], start=True, stop=True)
                nc.vector._custom_dve(
                    OP_A0F, out=a0[:, lo], in0=h0[:, lo], in1=A0n, s0=c0c
                )
            e2 = work.tile([128, ST], F16, tag="e2", name=f"e2_{s}")
            nc.vector.tensor_mul(e2, a0, w)
            state[s] = (cols, h0p, w, e1, a0, e2)

        def tail(s):
            cols, h0p, w, e1, a0, e2 = state.pop(s)
            fmh = []
            for h in range(2):
                lo = slice(h * TW, (h + 1) * TW)
                fm = ps.tile([128, TW], F32, tag="fm", bufs=2, name=f"fm_{s}_{h}")
                nc.tensor.matmul(
                    fm[0:8, :], KDT_sb, XTF[:, cols[h]], start=True, stop=True
                )
                nc.tensor.matmul(
                    fm[32:64, :], W0p_sb, a0[:, lo],
                    start=True, stop=True, tile_position=(0, 32),
                )
                nc.tensor.matmul(
                    fm[64:96, :], hvW_sb[:, 0:32], e1[:, lo],
                    start=True, stop=False, tile_position=(0, 64),
                )
                nc.tensor.matmul(
                    fm[64:96, :], hvW_sb[:, 32:64], e2[:, lo],
                    start=False, stop=True, tile_position=(0, 64),
                )
                fmh.append(fm)

            E = work.tile([128, ST], F16, tag="E", name=f"E_{s}")
            for h in range(2):
                lo = slice(h * TW, (h + 1) * TW)
                nc.scalar.copy(E[0:96, lo], fmh[h][0:96, :])
            bm = ps.tile([128, NCH * 96], F16, tag="a", bufs=2, name=f"bm_{s}")
            for c in range(NCH):
                nc.tensor.transpose(
                    bm[:, 96 * c : 96 * (c + 1)],
                    E[0:96, 128 * c : 128 * (c + 1)],
                    identH[0:96, 0:96],
                )
            tl = work.tile([128, NCH * 24], F32, tag="tl", name=f"tl_{s}")
            bmsrc = bass.AP(
                tensor=bm.tensor,
                offset=bm.offset,
                ap=[list(bm.ap[0]), [96, NCH], [32, 3], [1, DIM]],
            )
            tl4 = tl.rearrange("p (c q f) -> p c q f", q=3, f=DIM)
            nc.scalar.copy(tl4, bmsrc)

            p3 = tl4[:, :, 0, :]
            g3 = tl4[:, :, 1, :]
            hv1 = tl4[:, :, 2, 0:1]

            # strips hold -p, +g, -hvv (host-side sign flips), so the tail is
            # all plain ops: num' = -g.p - hvv = -num; su = g*(num'*rec) = -g*s
            nc.gpsimd.tensor_mul(Qv[:, :, 0, 0:8], g3, g3)
            nc.gpsimd.tensor_mul(Qv[:, :, 1, 0:8], g3, p3)
            nc.gpsimd.tensor_copy(Qv[:, :, 1, 8:9], hv1)
            R = work.tile([128, NCH * 2], F32, tag="R", name=f"R_{s}")
            Rv = R.rearrange("p (c g) -> p c g", g=2)
            nc.vector.tensor_reduce(Rv, Qv, axis=AX.X, op=OP.add)

            rec = work.tile([128, NCH], F32, tag="rec", name=f"rec_{s}")
            nc.vector.reciprocal(rec, Rv[:, :, 0:1].rearrange("p c g -> p (c g)"))
            s4 = work.tile([128, NCH], F32, tag="s4", name=f"s4_{s}")
            nc.gpsimd.tensor_mul(
                s4, Rv[:, :, 1:2].rearrange("p c g -> p (c g)"), rec
            )
            s4b = bass.AP(
                tensor=s4.tensor,
                offset=s4.offset,
                ap=[list(s4.ap[0]), [1, NCH], [0, DIM]],
            )
            su = work.tile([128, NCH * DIM], F32, tag="su", name=f"su_{s}")
            su3 = su.rearrange("p (c f) -> p c f", f=DIM)
            nc.gpsimd.tensor_mul(su3, g3, s4b)
            ob = out_sb[:, NCH * DIM * s : NCH * DIM * (s + 1)]
            ob3 = ob.rearrange("p (c f) -> p c f", f=DIM)
            nc.gpsimd.tensor_sub(ob3, su3, p3)  # out = su - (-p)

            oap = out[:, :]
            dst = bass.AP(
                tensor=oap.tensor,
                offset=oap.offset + s * ST * DIM,
                ap=[[DIM, 128], [128 * DIM, NCH], [1, DIM]],
            )
            nc.sync.dma_start(out=dst, in_=ob3)

        for s in range(NST):
            front(s)
            if s > 0:
                tail(s - 1)
        tail(NST - 1)

    if not nc.is_finalized():
        nc.finalize()

    return nc


_NC_CACHE = None


def _install_ntff_shim():
    """Register the axon NTFF profile hook (missing antenv.axon_hooks shim)."""
    import sys
    import types

    if "antenv.axon_hooks" in sys.modules:
        return
    try:
        sys.path.insert(0, "/root/.axon_site")
        from trn_agent_boot.trn_boot import _ntff_profile_via_ctypes

        hook = _ntff_profile_via_ctypes("/opt/axon/libaxon_pjrt.so")
        mod = types.ModuleType("antenv.axon_hooks")
        mod.get_axon_ntff_profile_hook = lambda: hook
        sys.modules["antenv.axon_hooks"] = mod
    except Exception:
        pass


def kernel(**inputs):
    global LAST_RESULTS, _NC_CACHE
    trace = bool(int(os.environ.get("KERNEL_TRACE", "0")))
    if trace:
        _install_ntff_shim()
    if _NC_CACHE is None:
        _NC_CACHE = build_nc()
    nc = _NC_CACHE

    X = np.ascontiguousarray(inputs["X"], dtype=np.float32)
    K = np.asarray(inputs["K"], np.float32)
    D = np.asarray(inputs["D"], np.float32)
    W0 = np.asarray(inputs["W0"], np.float32)
    W1 = np.asarray(inputs["W1"], np.float32)
    W2 = np.asarray(inputs["W2"], np.float32)
    w2 = W2.reshape(H)

    wrow = np.zeros((80, H), np.float32)
    wrow[0:8] = W0.T
    wrow[32:40] = W0.T
    wrow[64:80, 0:8] = np.concatenate([K.T, D.T], axis=0)  # p-strip = -p
    wa = -(W1 * w2[:, None])
    w0p = np.zeros((H, 32), np.float32)
    w0p[:, 0:8] = -W0          # g-strip = +g (a0' is -a0)
    hvw = np.zeros((H, 64), np.float32)
    hvw[:, 0:8] = (2.0 * w2)[:, None]   # hv-strip = -hvv
    hvw[:, 32:40] = -2.0
    cbm = np.zeros((H, 3), np.float32)
    cbm[:, 0] = np.asarray(inputs["b0"], np.float32)
    cbm[:, 1] = np.asarray(inputs["b1"], np.float32)
    cbm[:, 2] = W1.T @ w2

    shared = {
        "Wrow": wrow.astype(np.float16),
        "W1T": np.ascontiguousarray(W1.T).astype(np.float16),
        "Wa": wa.astype(np.float16),
        "W0p": w0p.astype(np.float16),
        "hvW": hvw.astype(np.float16),
        "cb": cbm,
    }
    in_maps = []
    for i in range(NCORES):
        xc = X[i * BC : (i + 1) * BC]
        xt = np.ascontiguousarray(xc.T).astype(np.float16)  # [16, BC]
        xtd = np.concatenate([xt[0:8], xt[8:16], xt], axis=0)  # [32, BC]
        m = {"XTd": np.ascontiguousarray(xtd)}
        m.update(shared)
        in_maps.append(m)

    res = run_bass_kernel_spmd(
        nc, in_maps, core_ids=list(range(NCORES)), trace=trace
    )
    LAST_RESULTS = res
    out_full = np.concatenate([res.results[i]["out"] for i in range(NCORES)], axis=0)
    return out_full.astype(np.float32)
